# revision 1
# baseline (speedup 1.0000x reference)
"""ContextQueryAttention (BiDAF-style) Trainium2 kernel.

Problem: nn_ContextQueryAttention_44066364457466
  query [B=8, Q=512, D=512], context [B=8, C=2048, D=512],
  query_weights/context_weights [D,1], dot_weights [D,D], mask all-True.
  out [B, C, 4D]: concat(context, c2q@query, context*that, context*qtc)

Sharding: data-parallel over batch. B == 8 == n_cores, one batch element
per NeuronCore, no collectives.

Math (per batch element, mask is all-ones so it drops out):
  A^T = W^T.. AT[e,c] = sum_d W[d,e] ctx[c,d]          (f32r matmuls)
  sim[c,q] = sum_e AT[e,c] qT[e,q] + cw[c] + qw[q]
  c2q   = softmax_q(sim) : E = exp(sim - m_c), s_c = sum_q E, c2q = E/s_c
  q2c   = softmax_c(sim) : F = exp(sim - M_q), S_q = sum_c F, q2c = F/S_q
  ctq   = c2q @ query
  qtc   = c2q @ (q2c^T @ context)      <- reassociated; [C,C] never built
The 1/s_c and 1/S_q scales are folded into matmul epilogues.

All heavy matmuls use float32r (rounded fp32, 1 cycle/row at N>=256 vs 4
for fp32; measured rel err ~1.5e-4 at K=512 which is far inside the
output tolerance). Partition-axis reductions (softmax over c; row stats
needed in the transposed orientation) are done via PE-transpose + free-
axis reduce, and via tiny ones-vector matmuls for the column sums.
"""

import numpy as np

B, Q, C, D = 8, 512, 2048, 512
P = 128
QT, CT, DT = Q // P, C // P, D // P  # 4, 16, 4
N_CORES = 8

_NC_CACHE = {}


def ds(start, size):
    return slice(start, start + size)


def _emit_body(nc, tc, pools, aps):
    import concourse.mybir as mybir

    f32 = mybir.dt.float32
    f32r = mybir.dt.float32r
    Exp = mybir.ActivationFunctionType.Exp
    Add = mybir.AluOpType.add
    Mult = mybir.AluOpType.mult
    AxX = mybir.AxisListType.X

    (constp, statp, persist, bigp, qfam, ldr, epool, stagep,
     ps_mm, ps_tr, ps_st) = pools
    (q_r3, c_r3, w_r3, cw_r3, qw_r3, out_r3) = aps

    id_f = constp.tile([P, P], f32, name="id_f", tag="id_f")
    from concourse.masks import make_identity
    make_identity(nc, id_f)
    id_r = constp.tile([P, P], f32r, name="id_r", tag="id_r")
    nc.vector.tensor_copy(id_r, id_f)
    ones_f = constp.tile([P, 1], f32, name="ones_f", tag="ones_f")
    nc.vector.memset(ones_f, 1.0)
    ones_r = constp.tile([P, 2], f32r, name="ones_r", tag="ones_r")
    nc.vector.tensor_copy(ones_r, ones_f.to_broadcast([P, 2]))
    ones_row_f = constp.tile([1, P], f32, name="ones_row_f", tag="ones_row_f")
    nc.vector.memset(ones_row_f, 1.0)
    ones_row_r = constp.tile([1, P], f32r, name="ones_row_r", tag="ones_row_r")
    nc.vector.tensor_copy(ones_row_r, ones_row_f)

    # ---- loads + rounds to f32r (through a small rotating loader) ----
    ctx_r = persist.tile([P, CT, D], f32r, name="ctx_r", tag="ctx_r")
    for i in range(4):
        ldt = ldr.tile([P, 4, D], f32, name="ldt", tag="ldr")
        nc.sync.dma_start(ldt, c_r3[:, ds(i * 4, 4), :])
        nc.vector.tensor_copy(ctx_r[:, ds(i * 4, 4), :], ldt)
    ctx_v = ctx_r.bitcast(f32)  # rounded context, read as f32 for elementwise

    q_f = ldr.tile([P, QT, D], f32, name="ldt", tag="ldr")
    nc.sync.dma_start(q_f, q_r3)
    q_r = persist.tile([P, QT, D], f32r, name="q_r", tag="q_r")
    nc.vector.tensor_copy(q_r, q_f)

    w_f = ldr.tile([P, DT, D], f32, name="ldt", tag="ldr")
    nc.sync.dma_start(w_f, w_r3)
    w_r = qfam.tile([P, DT, D], f32r, name="w_r", tag="qfam")
    nc.vector.tensor_copy(w_r, w_f)

    cwqw_f = statp.tile([P, DT, 2], f32, name="cwqw_f", tag="cwqw_f")
    nc.sync.dma_start(cwqw_f[:, :, 0:1], cw_r3)
    nc.sync.dma_start(cwqw_f[:, :, 1:2], qw_r3)
    cwqw_r = statp.tile([P, DT, 2], f32r, name="cwqw_r", tag="cwqw_r")
    nc.vector.tensor_copy(cwqw_r, cwqw_f)

    # ---- transposes: qT [d, q], cT [d, c] (f32r transposes, 1.5 cyc/row) ----
    qT_r = qfam.tile([P, DT, Q], f32r, name="qT_r", tag="qfam")
    for qt in range(QT):
        for dt in range(DT):
            tr = ps_tr.tile([P, P], f32r, name="tr", tag="tr")
            nc.tensor.transpose(tr, q_r[:, qt, ds(dt * P, P)], id_r)
            nc.vector.tensor_copy(qT_r[:, dt, ds(qt * P, P)], tr)

    cT_r = bigp.tile([P, DT, C], f32r, name="cT_r", tag="big")
    for ct in range(CT):
        for dt in range(DT):
            tr = ps_tr.tile([P, P], f32r, name="tr", tag="tr")
            nc.tensor.transpose(tr, ctx_r[:, ct, ds(dt * P, P)], id_r)
            nc.vector.tensor_copy(cT_r[:, dt, ds(ct * P, P)], tr)

    # ---- qw_row [1, Q] ----
    qw_row = statp.tile([1, Q], f32r, name="qw_row", tag="qw_row")
    pqw = ps_st.tile([1, Q], f32, name="pst", tag="st")
    for dt in range(DT):
        nc.tensor.matmul(pqw, cwqw_r[:, dt, 1:2], qT_r[:, dt, :],
                         start=(dt == 0), stop=(dt == DT - 1))
    nc.vector.tensor_copy(qw_row, pqw)

    # ---- cw_col [128, CT] ----
    cw_col = statp.tile([P, CT], f32, name="cw_col", tag="cw_col")
    for ct in range(CT):
        pcw = ps_st.tile([P, 2], f32, name="pst", tag="st")
        # N=1 f32r matmuls fail the ISA check; use both weight columns (N=2)
        for dt in range(DT):
            nc.tensor.matmul(pcw, cT_r[:, dt, ds(ct * P, P)], cwqw_r[:, dt, :],
                             start=(dt == 0), stop=(dt == DT - 1))
        nc.vector.tensor_copy(cw_col[:, ds(ct, 1)], pcw[:, 0:1])

    # ---- AT [e, c] = sum_d W[d,e] ctxT[d,c] ----
    AT_r = bigp.tile([P, DT, C], f32r, name="AT_r", tag="big")
    for et in range(DT):
        for cn in range(4):
            pm = ps_mm.tile([P, 512], f32, name="pm", tag="mm")
            for dt in range(DT):
                nc.tensor.matmul(pm, w_r[:, dt, ds(et * P, P)],
                                 cT_r[:, dt, ds(cn * 512, 512)],
                                 start=(dt == 0), stop=(dt == DT - 1))
            nc.vector.tensor_copy(AT_r[:, et, ds(cn * 512, 512)], pm)

    # ---- sim[c,q] = wcq + cw[c] + qw[q]; negm = -rowmax ----
    sim_t = persist.tile([P, CT, Q], f32r, name="sim_t", tag="sim")
    sim = sim_t.bitcast(f32)   # f32 view for non-matmul readers
    f2_r = sim_t               # after phase E, holds exp(sim - Mq) rounded
    negm = statp.tile([P, CT], f32, name="negm", tag="negm")
    for ct in range(CT):
        pm = ps_mm.tile([P, 512], f32, name="pm", tag="mm")
        for et in range(DT):
            nc.tensor.matmul(pm, AT_r[:, et, ds(ct * P, P)], qT_r[:, et, :],
                             start=(et == 0), stop=False)
        # += qw[q] broadcast over rows: K=1 ones matmul into the same bank
        nc.tensor.matmul(pm, ones_row_r, qw_row, start=False, stop=True)
        nc.vector.tensor_scalar_add(sim_t[:, ct, :], pm, cw_col[:, ds(ct, 1)])
        nc.vector.tensor_reduce(negm[:, ds(ct, 1)], sim[:, ct, :], axis=AxX,
                                op=mybir.AluOpType.max, negate=True)

    # ---- fused per-ct: Mq transposes (gate the long q2c chain) + c2q E/ET ----
    s_col = statp.tile([P, CT], f32, name="s_col", tag="s_col")
    ET_r = bigp.tile([P, QT, C], f32r, name="ET_r", tag="big")
    mq_parts = statp.tile([P, QT, CT], f32, name="mq_parts", tag="mq_parts")
    mq_stk = statp.tile([P, QT], f32, name="mq_stk", tag="mq_stk")
    mq_row = statp.tile([1, Q], f32r, name="mq_row", tag="mq_row")
    for ct in range(CT):
        e_t = epool.tile([P, Q], f32, name="e_t", tag="e_t")
        nc.scalar.activation(e_t, sim[:, ct, :], Exp, bias=negm[:, ds(ct, 1)],
                             scale=1.0, accum_out=s_col[:, ds(ct, 1)])
        for qt in range(QT):
            # Mq transpose first: it gates sim -> Mq -> F2 -> S_q -> G -> qtc
            trm = ps_tr.tile([P, P], f32, name="trm", tag="tr")
            nc.tensor.transpose(trm, sim[:, ct, ds(qt * P, P)], id_f)
            nc.vector.tensor_reduce(mq_parts[:, qt, ds(ct, 1)], trm, axis=AxX,
                                    op=mybir.AluOpType.max)
            tr = ps_tr.tile([P, P], f32, name="tr", tag="tr")
            nc.tensor.transpose(tr, e_t[:, ds(qt * P, P)], id_f)
            nc.vector.tensor_copy(ET_r[:, qt, ds(ct * P, P)], tr)
    r_col = statp.tile([P, CT], f32, name="r_col", tag="r_col")
    nc.vector.reciprocal(r_col, s_col)
    for qt in range(QT):
        nc.vector.tensor_reduce(mq_stk[:, ds(qt, 1)], mq_parts[:, qt, :],
                                axis=AxX, op=mybir.AluOpType.max, negate=True)
        ptr = ps_st.tile([1, P], f32, name="pst", tag="st")
        nc.tensor.transpose(ptr, mq_stk[:, ds(qt, 1)], id_f)
        nc.vector.tensor_copy(mq_row[0:1, ds(qt * P, P)], ptr)

    # ---- F2 = exp(sim - Mq[q]) in place (rounded); S_q via ones matmul ----
    # broadcast -Mq over all partitions with a K=1 ones matmul
    nmq_full = statp.tile([P, Q], f32, name="nmq_full", tag="nmq_full")
    pbc = ps_st.tile([P, Q], f32, name="pst", tag="st")
    nc.tensor.matmul(pbc, ones_row_r, mq_row, start=True, stop=True)
    nc.vector.tensor_copy(nmq_full, pbc)
    for ct in range(CT):
        sub_t = epool.tile([P, Q], f32, name="sub_t", tag="e_t")
        nc.vector.tensor_add(sub_t, sim[:, ct, :], nmq_full)
        nc.scalar.activation(f2_r[:, ct, :], sub_t, Exp)
    sq_col = statp.tile([P, QT], f32, name="sq_col", tag="sq_col")
    for qt in range(QT):
        pst = ps_st.tile([P, 2], f32, name="pst", tag="st")
        for ct in range(CT):
            nc.tensor.matmul(pst, f2_r[:, ct, ds(qt * P, P)], ones_r,
                             start=(ct == 0), stop=(ct == CT - 1))
        nc.vector.tensor_copy(sq_col[:, ds(qt, 1)], pst[:, 0:1])
    rq_col = statp.tile([P, QT], f32, name="rq_col", tag="rq_col")
    nc.vector.reciprocal(rq_col, sq_col)

    # ---- G[q,d] = (1/S_q) sum_c F2[c,q] ctx[c,d] ----
    G_r = qfam.tile([P, QT, D], f32r, name="G_r", tag="qfam")
    for qt in range(QT):
        pm = ps_mm.tile([P, 512], f32, name="pm", tag="mm")
        for ct in range(CT):
            nc.tensor.matmul(pm, f2_r[:, ct, ds(qt * P, P)], ctx_r[:, ct, :],
                             start=(ct == 0), stop=(ct == CT - 1))
        nc.scalar.mul(G_r[:, qt, :], pm, rq_col[:, ds(qt, 1)])

    # ---- outputs per c-tile ----
    for ct in range(CT):
        pc = ps_mm.tile([P, 512], f32, name="pm", tag="mm")
        for qt in range(QT):
            nc.tensor.matmul(pc, ET_r[:, qt, ds(ct * P, P)], q_r[:, qt, :],
                             start=(qt == 0), stop=(qt == QT - 1))
        pq = ps_mm.tile([P, 512], f32, name="pm", tag="mm")
        for qt in range(QT):
            nc.tensor.matmul(pq, ET_r[:, qt, ds(ct * P, P)], G_r[:, qt, :],
                             start=(qt == 0), stop=(qt == QT - 1))
        st = stagep.tile([P, 3 * D], f32, name="st", tag="stage")
        # block2 = ctq = r_c * (E @ query)
        nc.scalar.mul(st[:, 0:D], pc, r_col[:, ds(ct, 1)])
        # block3 = context * ctq
        nc.vector.tensor_mul(st[:, ds(D, D)], st[:, 0:D], ctx_v[:, ct, :])
        # block4 = context * qtc,  qtc = r_c * (E @ G)
        nc.vector.scalar_tensor_tensor(st[:, ds(2 * D, D)], pq,
                                       r_col[:, ds(ct, 1)], ctx_v[:, ct, :],
                                       op0=Mult, op1=Mult)
        nc.sync.dma_start(out_r3[:, ct, ds(D, 3 * D)], st)
        # block1 = context (exact f32, DRAM->DRAM)
        nc.sync.dma_start(out_r3[:, ct, 0:D], c_r3[:, ct, :])


def _build_bass(loop_n=1):
    import concourse.bass as bass  # noqa: F401
    import concourse.mybir as mybir
    import concourse.tile as tile
    from concourse import bacc

    f32 = mybir.dt.float32

    nc = bacc.Bacc("TRN2", debug=False, num_devices=N_CORES)
    q_d = nc.dram_tensor("query", [Q, D], f32, kind="ExternalInput")
    c_d = nc.dram_tensor("context", [C, D], f32, kind="ExternalInput")
    qw_d = nc.dram_tensor("query_weights", [D, 1], f32, kind="ExternalInput")
    cw_d = nc.dram_tensor("context_weights", [D, 1], f32, kind="ExternalInput")
    w_d = nc.dram_tensor("dot_weights", [D, D], f32, kind="ExternalInput")
    out_d = nc.dram_tensor("out", [C, 4 * D], f32, kind="ExternalOutput")

    aps = (
        q_d.ap().rearrange("(t p) d -> p t d", p=P),
        c_d.ap().rearrange("(t p) d -> p t d", p=P),
        w_d.ap().rearrange("(t p) e -> p t e", p=P),
        cw_d.ap().rearrange("(t p) o -> p t o", p=P),
        qw_d.ap().rearrange("(t p) o -> p t o", p=P),
        out_d.ap().rearrange("(t p) f -> p t f", p=P),
    )

    with tile.TileContext(nc) as tc:
        with (
            tc.tile_pool(name="const", bufs=1) as constp,
            tc.tile_pool(name="stats", bufs=1) as statp,
            tc.tile_pool(name="persist", bufs=1) as persist,
            tc.tile_pool(name="big", bufs=2) as bigp,
            tc.tile_pool(name="qfam", bufs=2) as qfam,
            tc.tile_pool(name="ldr", bufs=1) as ldr,
            tc.tile_pool(name="epool", bufs=3) as epool,
            tc.tile_pool(name="stage", bufs=2) as stagep,
            tc.tile_pool(name="ps_mm", bufs=4, space="PSUM") as ps_mm,
            tc.tile_pool(name="ps_tr", bufs=3, space="PSUM") as ps_tr,
            tc.tile_pool(name="ps_st", bufs=1, space="PSUM") as ps_st,
        ):
            pools = (constp, statp, persist, bigp, qfam, ldr, epool,
                     stagep, ps_mm, ps_tr, ps_st)
            if loop_n > 1:
                with tc.For_i(0, loop_n, 1):
                    _emit_body(nc, tc, pools, aps)
            else:
                _emit_body(nc, tc, pools, aps)
    nc.compile()
    return nc


def get_nc(loop_n=1):
    if loop_n not in _NC_CACHE:
        _NC_CACHE[loop_n] = _build_bass(loop_n)
    return _NC_CACHE[loop_n]


def kernel(query, context, query_weights, context_weights, dot_weights,
           mask=None):
    from concourse.bass_utils import run_bass_kernel_spmd

    query = np.ascontiguousarray(np.asarray(query, dtype=np.float32))
    context = np.ascontiguousarray(np.asarray(context, dtype=np.float32))
    query_weights = np.ascontiguousarray(np.asarray(query_weights, dtype=np.float32))
    context_weights = np.ascontiguousarray(np.asarray(context_weights, dtype=np.float32))
    dot_weights = np.ascontiguousarray(np.asarray(dot_weights, dtype=np.float32))
    # mask is all-True per the problem spec; NEG_INF * (~mask) == 0, so it
    # drops out of the computation entirely.

    nc = get_nc()
    in_maps = [
        {
            "query": query[b],
            "context": context[b],
            "query_weights": query_weights,
            "context_weights": context_weights,
            "dot_weights": dot_weights,
        }
        for b in range(B)
    ]
    res = run_bass_kernel_spmd(nc, in_maps, core_ids=list(range(N_CORES)))
    out = np.stack([res.results[b]["out"] for b in range(B)], axis=0)
    return np.ascontiguousarray(out.astype(np.float32))


if __name__ == "__main__":
    rng = np.random.default_rng(0)
    inputs = {
        "query": rng.standard_normal((B, Q, D), dtype=np.float32),
        "context": rng.standard_normal((B, C, D), dtype=np.float32),
        "query_weights": rng.standard_normal((D, 1), dtype=np.float32) * 0.05,
        "context_weights": rng.standard_normal((D, 1), dtype=np.float32) * 0.05,
        "dot_weights": rng.standard_normal((D, D), dtype=np.float32) * 0.05,
        "mask": np.ones((B, C, Q), dtype=bool),
    }
    out = kernel(**inputs)
    print("out", out.shape, out.dtype)

